# revision 1
# baseline (speedup 1.0000x reference)
"""Multi-head attention (16 heads, d_model=1024, B=2, T=S=2048) on 8 trn2 cores.

Strategy: tensor-parallel over heads — 2 heads per core. Each core:
  - projects Q (scaled by 1/8, +bq), K (bk dropped: softmax shift-invariant),
    V (bv folded into a host-side constant) for its 2 heads,
  - computes scores^T[s,t] = K_h @ (Q_h/8)^T + bias^T in PSUM,
  - exp via ScalarE (no max subtraction needed: scores are O(6)),
  - ctx^T[d,t] = sum_s V[s,d]·P^T[s,t] with an appended ones column giving the
    softmax denominator for free, normalized via reciprocal + partition
    broadcast,
  - out_partial[t,:] = ctx_n^T.T @ Wo[:,slice]^T.
Host: transposes/casts inputs to bf16 (activations + bias), sums the 8 partial
outputs, adds bo + bv@Wo.T.
"""

import sys

sys.path.insert(0, "/opt/trn_rl_repo")

from collections import deque
from contextlib import ExitStack

import ml_dtypes
import numpy as np

from concourse import bacc, mybir
from concourse.bass import ts
from concourse.bass_utils import run_bass_kernel_spmd
from concourse.tile import TileContext
from concourse.tile_rust import add_dep_helper

B, T, S, D, H, HD = 2, 2048, 2048, 1024, 16, 64
NCORES = 8
HPC = H // NCORES  # 2 heads per core
DPC = HPC * HD  # 128 head-dims per core
DCH = D // 128  # 8 dmodel chunks
NST = S // 128  # 16 s-tiles
TCH = 1024  # t-chunk width for score tiles
NTCH = T // TCH  # 2
BF = mybir.dt.bfloat16
F32 = mybir.dt.float32
EXP = mybir.ActivationFunctionType.Exp
COPY = mybir.ActivationFunctionType.Copy
ADD = mybir.AluOpType.add
MULT = mybir.AluOpType.mult

_PROGRAM = None


def build_program():
    nc = bacc.Bacc()
    qT = nc.declare_dram_parameter("qT", [B, D, T], BF, isOutput=False)
    kT = nc.declare_dram_parameter("kT", [B, D, S], BF, isOutput=False)
    vT = nc.declare_dram_parameter("vT", [B, D, S], BF, isOutput=False)
    biasT = nc.declare_dram_parameter("biasT", [B, HPC, S, T], BF, isOutput=False)
    wqT = nc.declare_dram_parameter("wqT", [D, DPC], BF, isOutput=False)
    wkT = nc.declare_dram_parameter("wkT", [D, DPC], BF, isOutput=False)
    wvT = nc.declare_dram_parameter("wvT", [D, DPC], BF, isOutput=False)
    woT = nc.declare_dram_parameter("woT", [DPC, D], BF, isOutput=False)
    bq_d = nc.declare_dram_parameter("bq", [DPC, 1], F32, isOutput=False)
    outp = nc.declare_dram_parameter("outp", [B, T, D], BF, isOutput=True)

    with TileContext(nc) as tc, ExitStack() as ctx:
        consts = ctx.enter_context(tc.tile_pool(name="consts", bufs=1))
        qkv_pool = ctx.enter_context(tc.tile_pool(name="qkv", bufs=6))
        vsb_pool = ctx.enter_context(tc.tile_pool(name="vsb", bufs=2))
        bias_pool = ctx.enter_context(tc.tile_pool(name="bias", bufs=16))
        pt_pool = ctx.enter_context(tc.tile_pool(name="pt", bufs=16))
        norm_pool = ctx.enter_context(tc.tile_pool(name="norm", bufs=3))
        ctxt_pool = ctx.enter_context(tc.tile_pool(name="ctxt", bufs=2))
        outs_pool = ctx.enter_context(tc.tile_pool(name="outs", bufs=4))
        # PSUM banks: shared sc tag 4 + ctx half tags 4 = 8
        ps512 = ctx.enter_context(tc.tile_pool(name="ps512", bufs=2, space="PSUM"))
        ctx_ps = ctx.enter_context(tc.tile_pool(name="ctx_ps", bufs=1, space="PSUM"))

        # weights, loaded once: [128, DCH, 128] with partition = dm within chunk
        wq_sb = consts.tile([128, DCH, DPC], BF, tag="wq")
        wk_sb = consts.tile([128, DCH, DPC], BF, tag="wk")
        wv_sb = consts.tile([128, DCH, DPC], BF, tag="wv")
        wo_sb = consts.tile([DPC, D], BF, tag="wo")
        bq_sb = consts.tile([DPC, 1], F32, tag="bq")
        for w_sb, w_d in [(wq_sb, wqT), (wk_sb, wkT), (wv_sb, wvT)]:
            nc.sync.dma_start(
                out=w_sb, in_=w_d[:].rearrange("(c p) q -> p c q", p=128)
            )
        nc.sync.dma_start(out=wo_sb, in_=woT[:])
        nc.sync.dma_start(out=bq_sb, in_=bq_d[:])

        for b in range(B):
            # ---- Q/K projections -> QT_sb/KT_sb [128 qd, T] bf16 ----
            QT_sb = qkv_pool.tile([DPC, T], BF, tag="QT")
            KT_sb = qkv_pool.tile([DPC, S], BF, tag="KT")
            for tch2 in range(T // 512):
                qt_sb = qkv_pool.tile([128, DCH, 512], BF, tag="qks")
                nc.sync.dma_start(
                    out=qt_sb,
                    in_=qT[b].rearrange("(c p) t -> p c t", p=128)[
                        :, :, ts(tch2, 512)
                    ],
                )
                pq = ps512.tile([128, 512], F32, tag="sc", bufs=4)
                for c in range(DCH):
                    nc.tensor.matmul(
                        pq[:],
                        lhsT=wq_sb[:, c, :],
                        rhs=qt_sb[:, c, :],
                        start=(c == 0),
                        stop=(c == DCH - 1),
                    )
                # QT = (Q + bq) / 8  (attention scale folded in)
                nc.vector.tensor_scalar(
                    out=QT_sb[:, ts(tch2, 512)],
                    in0=pq[:],
                    scalar1=bq_sb[:],
                    scalar2=0.125,
                    op0=ADD,
                    op1=MULT,
                )
                kt_sb = qkv_pool.tile([128, DCH, 512], BF, tag="qks")
                nc.sync.dma_start(
                    out=kt_sb,
                    in_=kT[b].rearrange("(c p) t -> p c t", p=128)[
                        :, :, ts(tch2, 512)
                    ],
                )
                pk = ps512.tile([128, 512], F32, tag="sc", bufs=4)
                for c in range(DCH):
                    nc.tensor.matmul(
                        pk[:],
                        lhsT=wk_sb[:, c, :],
                        rhs=kt_sb[:, c, :],
                        start=(c == 0),
                        stop=(c == DCH - 1),
                    )
                nc.vector.tensor_copy(out=KT_sb[:, ts(tch2, 512)], in_=pk[:])

            # ---- V projection -> per s-tile [128 s, 130] (64+ones, 64+ones) ----
            v_tiles = []
            for st in range(NST):
                vt_sb = qkv_pool.tile([128, DCH, 128], BF, tag="vs")
                nc.sync.dma_start(
                    out=vt_sb,
                    in_=vT[b].rearrange("(c p) s -> p c s", p=128)[
                        :, :, ts(st, 128)
                    ],
                )
                pv = ps512.tile([128, 512], F32, tag="sc", bufs=4)
                for c in range(DCH):
                    nc.tensor.matmul(
                        pv[:, 0:DPC],
                        lhsT=vt_sb[:, c, :],
                        rhs=wv_sb[:, c, :],
                        start=(c == 0),
                        stop=(c == DCH - 1),
                    )
                v_sb = vsb_pool.tile([128, 2, HD + 1], BF, tag=f"v{st}")
                for h in range(HPC):
                    nc.vector.tensor_copy(
                        out=v_sb[:, h, 0:HD], in_=pv[:, ts(h, HD)]
                    )
                    nc.vector.memset(v_sb[:, h, HD : HD + 1], 1.0)
                v_tiles.append(v_sb)

            # ---- attention ----
            ctxT_sb = ctxt_pool.tile([DPC, T], BF, tag="ctxT")
            for tch in range(NTCH):
                for u in range(TCH // 512):
                    t0 = tch * TCH + u * 512
                    # Two PSUM accumulators per head: the s-contraction is
                    # split into rows 0-63 / 64-127 halves so each (h, st)
                    # pair of K=64 ctx matmuls runs concurrently in disjoint
                    # row groups; halves are summed during evacuation.
                    cps = [
                        [
                            ctx_ps.tile(
                                [128, 512], F32, tag=f"ctx{h}{a}", name=f"cps{h}{a}"
                            )
                            for a in range(2)
                        ]
                        for h in range(HPC)
                    ]
                    pend_ctx = deque(maxlen=8)
                    for st in range(NST):
                        bias_tiles = []
                        for h in range(HPC):
                            bias_sb = bias_pool.tile([128, 512], BF, tag="bias")
                            nc.sync.dma_start(
                                out=bias_sb,
                                in_=biasT[b, h, ts(st, 128), t0 : t0 + 512],
                            )
                            bias_tiles.append(bias_sb)
                        # The two heads' K=64 score matmuls are row-packed
                        # (rows 0-63 / 64-127) so they can run concurrently.
                        # Their tiles get dedicated slots (bufs=4) and boosted
                        # priority so the pair issues back-to-back on the PE
                        # instead of interleaving with ctx matmuls.
                        scs = []
                        sc_mms = []
                        with tc.high_priority(offset=400):
                            for h in range(HPC):
                                sc = ps512.tile([128, 512], F32, tag="sc", bufs=4)
                                mm = nc.tensor.matmul(
                                    sc[:],
                                    lhsT=KT_sb[ts(h, HD), ts(st, 128)],
                                    rhs=QT_sb[ts(h, HD), t0 : t0 + 512],
                                    start=True,
                                    stop=True,
                                )
                                scs.append(sc)
                                sc_mms.append(mm)
                        add_dep_helper(sc_mms[1].ins, sc_mms[0].ins, sync=False,
                                       reason="score pair adjacency")
                        for pc in list(pend_ctx):
                            add_dep_helper(pc.ins, sc_mms[1].ins, sync=False,
                                           reason="ctx yields to score pair")
                        for h in range(HPC):
                            pt = pt_pool.tile([128, 512], BF, tag="pt")
                            nc.scalar.activation(out=pt[:], in_=scs[h][:], func=EXP)
                            # attn_bias enters multiplicatively: host sends
                            # exp(bias), so this is an all-bf16 SBUF multiply
                            # (DVE fast mode) instead of an f32 PSUM add.
                            nc.vector.tensor_tensor(
                                out=pt[:], in0=pt[:], in1=bias_tiles[h][:], op=MULT
                            )
                            half_mms = []
                            for a in range(2):
                                cmm = nc.tensor.matmul(
                                    cps[h][a][0 : HD + 1, :],
                                    lhsT=v_tiles[st][ts(a, 64), h, :],
                                    rhs=pt[ts(a, 64), :],
                                    start=(st == 0),
                                    stop=(st == NST - 1),
                                )
                                half_mms.append(cmm)
                                pend_ctx.append(cmm)
                            add_dep_helper(
                                half_mms[1].ins,
                                half_mms[0].ins,
                                sync=False,
                                reason="ctx half pair adjacency",
                            )
                    for h in range(HPC):
                        # evacuate raw ctx+denom to SBUF (denominator into row
                        # 0: reciprocal_approx_fast needs base partition 0).
                        # This releases the PSUM tile so the next chunk's
                        # accumulation starts while normalization trails.
                        cu = norm_pool.tile([128, 512], F32, tag="cu", name=f"cu{h}")
                        nc.scalar.activation(
                            out=cu[0:1, :],
                            in_=cps[h][0][HD : HD + 1, :],
                            func=COPY,
                        )
                        nc.scalar.activation(
                            out=cu[64:128, :], in_=cps[h][0][0:HD, :], func=COPY
                        )
                        nc.vector.tensor_tensor(
                            out=cu[0:1, :],
                            in0=cps[h][1][HD : HD + 1, :],
                            in1=cu[0:1, :],
                            op=ADD,
                        )
                        nc.vector.tensor_tensor(
                            out=cu[64:128, :],
                            in0=cps[h][1][0:HD, :],
                            in1=cu[64:128, :],
                            op=ADD,
                        )
                        rd = norm_pool.tile([1, 512], F32, tag="rd")
                        nc.vector.reciprocal_approx_fast(out=rd[:], in_=cu[0:1, :])
                        rrep = norm_pool.tile([128, 512], F32, tag="rrep")
                        nc.gpsimd.partition_broadcast(rrep[:], rd[:])
                        nc.vector.tensor_tensor(
                            out=ctxT_sb[ts(h, HD), t0 : t0 + 512],
                            in0=cu[64:128, :],
                            in1=rrep[64:128, :],
                            op=MULT,
                        )

                # ---- out projection for this t-chunk ----
                for tt in range(TCH // 128):
                    t0 = tch * TCH + tt * 128
                    out_sb = outs_pool.tile([128, D], BF, tag="out")
                    for eh in range(D // 512):
                        po = ps512.tile([128, 512], F32, tag="sc", bufs=4)
                        nc.tensor.matmul(
                            po[:],
                            lhsT=ctxT_sb[:, t0 : t0 + 128],
                            rhs=wo_sb[:, ts(eh, 512)],
                            start=True,
                            stop=True,
                        )
                        nc.vector.tensor_copy(out=out_sb[:, ts(eh, 512)], in_=po[:])
                    nc.sync.dma_start(out=outp[b, t0 : t0 + 128, :], in_=out_sb)

    nc.compile()
    return nc


def _get_program():
    global _PROGRAM
    if _PROGRAM is None:
        _PROGRAM = build_program()
    return _PROGRAM


def make_in_maps(query, key, value, attn_bias, Wq, bq, Wk, Wv, Wo):
    bf = ml_dtypes.bfloat16
    f32 = np.float32
    qT = np.ascontiguousarray(np.asarray(query, f32).transpose(0, 2, 1)).astype(bf)
    kT = np.ascontiguousarray(np.asarray(key, f32).transpose(0, 2, 1)).astype(bf)
    vT = np.ascontiguousarray(np.asarray(value, f32).transpose(0, 2, 1)).astype(bf)
    attn_bias = np.asarray(attn_bias, f32)
    Wq, Wk, Wv, Wo = (np.asarray(w, f32) for w in (Wq, Wk, Wv, Wo))
    in_maps = []
    for c in range(NCORES):
        dsl = slice(DPC * c, DPC * (c + 1))
        hsl = slice(HPC * c, HPC * (c + 1))
        biasT = np.ascontiguousarray(
            np.exp(attn_bias[:, hsl]).transpose(0, 1, 3, 2)
        ).astype(bf)
        in_maps.append(
            {
                "qT": qT,
                "kT": kT,
                "vT": vT,
                "biasT": biasT,
                "wqT": np.ascontiguousarray(Wq[dsl].T).astype(bf),
                "wkT": np.ascontiguousarray(Wk[dsl].T).astype(bf),
                "wvT": np.ascontiguousarray(Wv[dsl].T).astype(bf),
                "woT": np.ascontiguousarray(Wo[:, dsl].T).astype(bf),
                "bq": np.ascontiguousarray(np.asarray(bq, f32)[dsl]).reshape(DPC, 1),
            }
        )
    return in_maps


def combine_outputs(results, Wo, bv, bo):
    out = np.zeros((B, T, D), np.float64)
    for c in range(NCORES):
        out += results[c]["outp"].astype(np.float64)
    const = np.asarray(bv, np.float64) @ np.asarray(Wo, np.float64).T + np.asarray(
        bo, np.float64
    )
    out += const
    return out.astype(np.float32)


def kernel(
    query,
    key,
    value,
    attn_bias,
    key_padding_mask,
    Wq,
    bq,
    Wk,
    bk,
    Wv,
    bv,
    Wo,
    bo,
):
    # key_padding_mask is all-False in this problem; bk is dropped (softmax is
    # invariant to a per-row constant shift); bv/bo enter via a host constant.
    nc = _get_program()
    in_maps = make_in_maps(query, key, value, attn_bias, Wq, bq, Wk, Wv, Wo)
    res = run_bass_kernel_spmd(nc, in_maps, list(range(NCORES)))
    return combine_outputs(res.results, Wo, bv, bo)


if __name__ == "__main__":
    rng = np.random.default_rng(0)
    args = {
        "query": rng.standard_normal((B, T, D), np.float32),
        "key": rng.standard_normal((B, S, D), np.float32),
        "value": rng.standard_normal((B, S, D), np.float32),
        "attn_bias": rng.standard_normal((B, H, T, S), np.float32),
        "key_padding_mask": np.zeros((B, S), bool),
        "Wq": rng.uniform(-0.03125, 0.03125, (D, D)).astype(np.float32),
        "bq": rng.uniform(-0.03125, 0.03125, D).astype(np.float32),
        "Wk": rng.uniform(-0.03125, 0.03125, (D, D)).astype(np.float32),
        "bk": rng.uniform(-0.03125, 0.03125, D).astype(np.float32),
        "Wv": rng.uniform(-0.03125, 0.03125, (D, D)).astype(np.float32),
        "bv": rng.uniform(-0.03125, 0.03125, D).astype(np.float32),
        "Wo": rng.uniform(-0.03125, 0.03125, (D, D)).astype(np.float32),
        "bo": rng.uniform(-0.03125, 0.03125, D).astype(np.float32),
    }
    out = kernel(**args)
    print("kernel ran, out shape", out.shape, "std", out.std())



# revision 4
# speedup vs baseline: 1.0900x; 1.0900x over previous
"""Multi-head attention (16 heads, d_model=1024, B=2, T=S=2048) on 8 trn2 cores.

Sharding: (batch, head-group) — core c handles batch c//4 and heads
[4*(c%4) : 4*(c%4)+4]. This halves per-core q/k/v reads (one batch: 12.6MB
vs 25.2MB) and the partial-output write (4.2MB vs 8.4MB) relative to
head-only sharding; host sums 4 partials per batch.

Per core:
  - project Q (scaled 1/8, +bq; bk dropped: softmax shift-invariant), K, V for
    its 4 heads from the batch's q/k/v (bf16, host pre-tiled for contiguous
    per-partition DMA descriptors),
  - scores^T[s,t] = K_h @ (Q_h/8)^T computed per (s-tile, head) into a 3-bank
    PSUM group; one wide-FD ACT exp (amortizes the ~352-cycle ACT overhead)
    evacuates 3 tiles at once,
  - attn_bias enters multiplicatively: host sends exp(bias) bf16 pre-tiled so
    each (t-chunk, head-pair) slab is ONE 4.2MB DMA with 32KB contiguous per
    partition; one wide bf16 DVE multiply per exp group,
  - ctx^T[d,t] = V^T @ P per (s-tile, head) as single K=128 matmuls (M=65:
    64 dims + ones column giving the softmax denominator), accumulated in one
    PSUM bank per head; normalized via reciprocal + partition broadcast,
  - out_partial[t,:] = ctx_n^T.T @ Wo[:,slice]^T.
Host: pre-tiles/casts inputs to bf16, sums the 4 partial outputs per batch,
adds bo + bv@Wo.T.
"""

import sys

sys.path.insert(0, "/opt/trn_rl_repo")

from collections import deque
from contextlib import ExitStack

import ml_dtypes
import numpy as np

from concourse import bacc, mybir
from concourse.bass import ts
from concourse.bass_utils import run_bass_kernel_spmd
from concourse.tile import TileContext
from concourse.tile_rust import add_dep_helper

B, T, S, D, H, HD = 2, 2048, 2048, 1024, 16, 64
NCORES = 8
HPC = 4  # heads per core
NPAIR = HPC // 2  # head pairs per core
DPC = HPC * HD  # 256 head-dims per core
DCH = D // 128  # 8 dmodel chunks
NST = S // 128  # 16 s-tiles
NT5 = T // 512  # 4 t-chunks
NSLOT = NST * 2  # 32 (s-tile, head-of-pair) slots per (t-chunk, pair)
GRP = 3  # psum banks / score tiles per exp group
BF = mybir.dt.bfloat16
F32 = mybir.dt.float32
EXP = mybir.ActivationFunctionType.Exp
ADD = mybir.AluOpType.add
MULT = mybir.AluOpType.mult

_PROGRAM = None


def build_program():
    nc = bacc.Bacc()
    qH = nc.declare_dram_parameter("qH", [NT5, 128, DCH, 512], BF, isOutput=False)
    kH = nc.declare_dram_parameter("kH", [NT5, 128, DCH, 512], BF, isOutput=False)
    vH = nc.declare_dram_parameter("vH", [NT5, 128, DCH, 512], BF, isOutput=False)
    biasH = nc.declare_dram_parameter(
        "biasH", [NT5, NPAIR, 128, NSLOT, 512], BF, isOutput=False
    )
    wqH = nc.declare_dram_parameter("wqH", [128, DCH, DPC], BF, isOutput=False)
    wkH = nc.declare_dram_parameter("wkH", [128, DCH, DPC], BF, isOutput=False)
    wvH = nc.declare_dram_parameter("wvH", [128, DCH, DPC], BF, isOutput=False)
    woH = nc.declare_dram_parameter("woH", [128, 2, D], BF, isOutput=False)
    bqH = nc.declare_dram_parameter("bqH", [128, 2], F32, isOutput=False)
    outp = nc.declare_dram_parameter("outp", [T, D], BF, isOutput=True)

    with TileContext(nc) as tc, ExitStack() as ctx:
        consts = ctx.enter_context(tc.tile_pool(name="consts", bufs=1))
        io_pool = ctx.enter_context(tc.tile_pool(name="io", bufs=4))
        qk_pool = ctx.enter_context(tc.tile_pool(name="qk", bufs=1))
        bias_pool = ctx.enter_context(tc.tile_pool(name="bias", bufs=2))
        pt_pool = ctx.enter_context(tc.tile_pool(name="pt", bufs=4))
        norm_pool = ctx.enter_context(tc.tile_pool(name="norm", bufs=2))
        outs_pool = ctx.enter_context(tc.tile_pool(name="outs", bufs=2))
        # PSUM: sc tag 2 bufs x 3 banks + cxpo tag 2 bufs x 1 bank = 8 banks
        psum = ctx.enter_context(tc.tile_pool(name="psum", bufs=2, space="PSUM"))

        wq_sb = consts.tile([128, DCH, DPC], BF, tag="wq")
        wk_sb = consts.tile([128, DCH, DPC], BF, tag="wk")
        wv_sb = consts.tile([128, DCH, DPC], BF, tag="wv")
        wo_sb = consts.tile([128, 2, D], BF, tag="wo")
        bq_sb = consts.tile([128, 2], F32, tag="bq")
        nc.sync.dma_start(out=wq_sb, in_=wqH[:])
        nc.sync.dma_start(out=wk_sb, in_=wkH[:])
        nc.sync.dma_start(out=wv_sb, in_=wvH[:])
        nc.sync.dma_start(out=wo_sb, in_=woH[:])
        nc.sync.dma_start(out=bq_sb, in_=bqH[:])

        # persistent activations
        QT_sb = qk_pool.tile([128, NPAIR, T], BF, tag="QT")
        KT_sb = qk_pool.tile([128, NPAIR, S], BF, tag="KT")
        # V in [s, head, dim] layout with a ones column at dim 64
        v_all = qk_pool.tile([128, NST, HPC, HD + 1], BF, tag="vall")
        nc.vector.memset(v_all[:, :, :, HD : HD + 1], 1.0)
        ctxT_sb = qk_pool.tile([128, NPAIR, T], BF, tag="ctxT")

        # ---- projections, interleaved per 512-chunk ----
        for c2 in range(NT5):
            kt = io_pool.tile([128, DCH, 512], BF, tag="stg")
            nc.sync.dma_start(out=kt, in_=kH[c2])
            pk = psum.tile([128, GRP * 512], F32, tag="sc", name="pk")
            for a in range(NPAIR):
                for c in range(DCH):
                    nc.tensor.matmul(
                        pk[:, ts(a, 512)],
                        lhsT=wk_sb[:, c, ts(a, 128)],
                        rhs=kt[:, c, :],
                        start=(c == 0),
                        stop=(c == DCH - 1),
                    )
            for a in range(NPAIR):
                nc.vector.tensor_copy(
                    out=KT_sb[:, a, ts(c2, 512)], in_=pk[:, ts(a, 512)]
                )

            vt = io_pool.tile([128, DCH, 512], BF, tag="stg")
            nc.sync.dma_start(out=vt, in_=vH[c2])
            for stl in range(4):
                st = c2 * 4 + stl
                pv = psum.tile([128, 512], F32, tag="cxpo", name="pv")
                for c in range(DCH):
                    nc.tensor.matmul(
                        pv[:, 0:DPC],
                        lhsT=vt[:, c, ts(stl, 128)],
                        rhs=wv_sb[:, c, :],
                        start=(c == 0),
                        stop=(c == DCH - 1),
                    )
                for h in range(HPC):
                    nc.vector.tensor_copy(
                        out=v_all[:, st, h, 0:HD], in_=pv[:, ts(h, HD)]
                    )

            qt = io_pool.tile([128, DCH, 512], BF, tag="stg")
            nc.sync.dma_start(out=qt, in_=qH[c2])
            pq = psum.tile([128, GRP * 512], F32, tag="sc", name="pq")
            for a in range(NPAIR):
                for c in range(DCH):
                    nc.tensor.matmul(
                        pq[:, ts(a, 512)],
                        lhsT=wq_sb[:, c, ts(a, 128)],
                        rhs=qt[:, c, :],
                        start=(c == 0),
                        stop=(c == DCH - 1),
                    )
            for a in range(NPAIR):
                # QT = (Q + bq) / 8  (attention scale folded in)
                nc.vector.tensor_scalar(
                    out=QT_sb[:, a, ts(c2, 512)],
                    in0=pq[:, ts(a, 512)],
                    scalar1=bq_sb[:, a : a + 1],
                    scalar2=0.125,
                    op0=ADD,
                    op1=MULT,
                )

        # ---- attention ----
        for t5 in range(NT5):
            t0 = t5 * 512
            for hp in range(NPAIR):
                bias_sb = bias_pool.tile([128, NSLOT, 512], BF, tag="bias")
                nc.sync.dma_start(out=bias_sb, in_=biasH[t5, hp])
                cxs = [
                    psum.tile([128, 512], F32, tag="cxpo", name=f"cx{h2}")
                    for h2 in range(2)
                ]
                pend_ctx = deque(maxlen=8)
                prev_mm = None
                ngrp = (NSLOT + GRP - 1) // GRP
                for g in range(ngrp):
                    gsz = min(GRP, NSLOT - g * GRP)
                    sc_big = psum.tile([128, GRP * 512], F32, tag="sc", name="sc")
                    for j in range(gsz):
                        st, h2 = divmod(g * GRP + j, 2)
                        with tc.high_priority(offset=400):
                            mm = nc.tensor.matmul(
                                sc_big[:, ts(j, 512)],
                                lhsT=KT_sb[ts(h2, HD), hp, ts(st, 128)],
                                rhs=QT_sb[ts(h2, HD), hp, t0 : t0 + 512],
                                start=True,
                                stop=True,
                            )
                        if h2 == 1:
                            add_dep_helper(
                                mm.ins, prev_mm.ins, sync=False,
                                reason="score pair adjacency",
                            )
                        prev_mm = mm
                    for pc in list(pend_ctx):
                        add_dep_helper(
                            pc.ins, prev_mm.ins, sync=False,
                            reason="ctx yields to score group",
                        )
                    # wide-FD exp of the whole group, then one bias multiply
                    pt = pt_pool.tile([128, GRP * 512], BF, tag="pt")
                    nc.scalar.activation(
                        out=pt[:, 0 : gsz * 512], in_=sc_big[:, 0 : gsz * 512],
                        func=EXP,
                    )
                    nc.vector.tensor_tensor(
                        out=pt[:, 0 : gsz * 512],
                        in0=pt[:, 0 : gsz * 512],
                        in1=bias_sb[:, g * GRP : g * GRP + gsz, :],
                        op=MULT,
                    )
                    for j in range(gsz):
                        st, h2 = divmod(g * GRP + j, 2)
                        cmm = nc.tensor.matmul(
                            cxs[h2][0 : HD + 1, :],
                            lhsT=v_all[:, st, hp * 2 + h2, :],
                            rhs=pt[:, ts(j, 512)],
                            start=(st == 0),
                            stop=(st == NST - 1),
                        )
                        pend_ctx.append(cmm)
                # evacuate + normalize (denominator in ctx row 64; move it to
                # partition 0 for reciprocal, ctx data to rows 64-127)
                for h2 in range(2):
                    cu = norm_pool.tile([128, 512], F32, tag="cu")
                    nc.vector.tensor_copy(out=cu[0:1, :], in_=cxs[h2][HD : HD + 1, :])
                    nc.vector.tensor_copy(out=cu[64:128, :], in_=cxs[h2][0:HD, :])
                    rd = norm_pool.tile([1, 512], F32, tag="rd")
                    nc.vector.reciprocal_approx_fast(out=rd[:], in_=cu[0:1, :])
                    rrep = norm_pool.tile([128, 512], F32, tag="rrep")
                    nc.gpsimd.partition_broadcast(rrep[:], rd[:])
                    nc.vector.tensor_tensor(
                        out=ctxT_sb[ts(h2, HD), hp, t0 : t0 + 512],
                        in0=cu[64:128, :],
                        in1=rrep[64:128, :],
                        op=MULT,
                    )

            # ---- out projection for this t-chunk ----
            out_sb = outs_pool.tile([128, 4, D], BF, tag="out")
            for tt in range(4):
                tb = t0 + tt * 128
                for eh in range(2):
                    po = psum.tile([128, 512], F32, tag="cxpo", name="po")
                    for a in range(NPAIR):
                        nc.tensor.matmul(
                            po[:],
                            lhsT=ctxT_sb[:, a, tb : tb + 128],
                            rhs=wo_sb[:, a, ts(eh, 512)],
                            start=(a == 0),
                            stop=(a == NPAIR - 1),
                        )
                    nc.vector.tensor_copy(out=out_sb[:, tt, ts(eh, 512)], in_=po[:])
            nc.sync.dma_start(
                out=outp[t0 : t0 + 512, :].rearrange("(tt p) d -> p tt d", p=128),
                in_=out_sb,
            )

    nc.compile()
    return nc


def _get_program():
    global _PROGRAM
    if _PROGRAM is None:
        _PROGRAM = build_program()
    return _PROGRAM


def make_in_maps(query, key, value, attn_bias, Wq, bq, Wk, Wv, Wo):
    bf = ml_dtypes.bfloat16
    f32 = np.float32

    def tile_act(x):  # [T, D] -> [NT5, 128p, DCH, 512t]
        v = np.asarray(x, f32).reshape(NT5, 512, DCH, 128)  # [t5, tt, c, p]
        return np.ascontiguousarray(v.transpose(0, 3, 2, 1)).astype(bf)

    def tile_w(w):  # rows of W for this core's dims: [DPC, D] -> [128p, DCH, DPC]
        v = np.asarray(w, f32).T.reshape(DCH, 128, DPC)  # [c, p, j]
        return np.ascontiguousarray(v.transpose(1, 0, 2)).astype(bf)

    acts = {}
    for b in range(B):
        acts[b] = (
            tile_act(np.asarray(query)[b]),
            tile_act(np.asarray(key)[b]),
            tile_act(np.asarray(value)[b]),
        )
    attn_bias = np.asarray(attn_bias, f32)
    Wq, Wk, Wv, Wo = (np.asarray(w, f32) for w in (Wq, Wk, Wv, Wo))
    bq = np.asarray(bq, f32)
    in_maps = []
    for c in range(NCORES):
        b, grp = divmod(c, NCORES // B)
        hsl = slice(grp * HPC, (grp + 1) * HPC)
        dsl = slice(grp * DPC, (grp + 1) * DPC)
        A = np.exp(attn_bias[b, hsl])  # [4h, T, S]
        A = A.reshape(NPAIR, 2, NT5, 512, NST, 128)  # [hp, h2, t5, tt, st, p]
        bH = np.ascontiguousarray(A.transpose(2, 0, 5, 4, 1, 3)).astype(bf)
        bH = bH.reshape(NT5, NPAIR, 128, NSLOT, 512)
        wo = Wo[:, dsl]  # [D, DPC]
        woH = np.ascontiguousarray(wo.T.reshape(2, 128, D).transpose(1, 0, 2)).astype(
            bf
        )
        in_maps.append(
            {
                "qH": acts[b][0],
                "kH": acts[b][1],
                "vH": acts[b][2],
                "biasH": bH,
                "wqH": tile_w(Wq[dsl]),
                "wkH": tile_w(Wk[dsl]),
                "wvH": tile_w(Wv[dsl]),
                "woH": woH,
                "bqH": np.ascontiguousarray(bq[dsl].reshape(2, 128).T),
            }
        )
    return in_maps


def combine_outputs(results, Wo, bv, bo):
    out = np.zeros((B, T, D), np.float64)
    per_b = NCORES // B
    for c in range(NCORES):
        out[c // per_b] += results[c]["outp"].astype(np.float64)
    const = np.asarray(bv, np.float64) @ np.asarray(Wo, np.float64).T + np.asarray(
        bo, np.float64
    )
    out += const
    return out.astype(np.float32)


def kernel(
    query,
    key,
    value,
    attn_bias,
    key_padding_mask,
    Wq,
    bq,
    Wk,
    bk,
    Wv,
    bv,
    Wo,
    bo,
):
    # key_padding_mask is all-False in this problem; bk is dropped (softmax is
    # invariant to a per-row constant shift); bv/bo enter via a host constant.
    nc = _get_program()
    in_maps = make_in_maps(query, key, value, attn_bias, Wq, bq, Wk, Wv, Wo)
    res = run_bass_kernel_spmd(nc, in_maps, list(range(NCORES)))
    return combine_outputs(res.results, Wo, bv, bo)


if __name__ == "__main__":
    rng = np.random.default_rng(0)
    args = {
        "query": rng.standard_normal((B, T, D), np.float32),
        "key": rng.standard_normal((B, S, D), np.float32),
        "value": rng.standard_normal((B, S, D), np.float32),
        "attn_bias": rng.standard_normal((B, H, T, S), np.float32),
        "key_padding_mask": np.zeros((B, S), bool),
        "Wq": rng.uniform(-0.03125, 0.03125, (D, D)).astype(np.float32),
        "bq": rng.uniform(-0.03125, 0.03125, D).astype(np.float32),
        "Wk": rng.uniform(-0.03125, 0.03125, (D, D)).astype(np.float32),
        "bk": rng.uniform(-0.03125, 0.03125, D).astype(np.float32),
        "Wv": rng.uniform(-0.03125, 0.03125, (D, D)).astype(np.float32),
        "bv": rng.uniform(-0.03125, 0.03125, D).astype(np.float32),
        "Wo": rng.uniform(-0.03125, 0.03125, (D, D)).astype(np.float32),
        "bo": rng.uniform(-0.03125, 0.03125, D).astype(np.float32),
    }
    out = kernel(**args)
    print("kernel ran, out shape", out.shape, "std", out.std())
